# revision 2
# baseline (speedup 1.0000x reference)
"""Trainium2 Bass kernel for DeeperGCN (nn_DeeperGCN_65369402245674), v2.

Changes vs v1 baseline (5.43ms):
  * bf16 edge pipeline: gather table, edge_attr, messages, exp/products and
    the identity-matmul segment accumulation all run in bf16 (PSUM fp32).
  * Gathers use prepare_only + trigger_dma on 4 rotating SWDGE queues so
    GPSIMD only pays descriptor *generation*; transfers overlap.
  * eps algebra: sum(e*(relu+eps))/sum(e) = sum(e*relu)/sum(e) + eps, with
    eps folded into b1 (b1' = b1 + eps*colsum(W1)).  Numerator needs no pad
    correction (pad lanes have relu=0).
  * AllGather between layers is bf16 and chunked (NCH chunks of windows,
    issued as soon as a chunk's node phase finishes) to overlap with compute.
    Table rows are laid out chunk-major to match AllGather concatenation.
  * Scalar engine runs ONLY Exp in the edge phase plus one Rsqrt per chunk
    (LayerNorm), everything else on DVE -> no activation-table thrash.
  * ea loads in 32-slot slabs (8KB/partition contiguous).
"""

import os
import numpy as np
import ml_dtypes

GPREP = os.environ.get("GPREP", "0") == "1"
SCRATCH = int(os.environ.get("SCRATCH", "32768"))

import concourse.bass as bass
import concourse.mybir as mybir
import concourse.tile as tile
from concourse import bacc
from concourse.bass_utils import run_bass_kernel_spmd
from concourse.masks import make_identity

P = 128
D = 128
L = 3
NCORES = 8
G = int(os.environ.get("GSIZE", "8"))  # slots per gather group
SLAB = 32 if G <= 8 else G  # slots per ea DMA slab
NCH = 4     # window chunks per layer for the chunked AllGather
EPS = 1e-7
LN_EPS = 1e-5
FP = mybir.dt.float32
BF = mybir.dt.bfloat16
I16 = mybir.dt.int16
AF = mybir.ActivationFunctionType
ALU = mybir.AluOpType
AX = mybir.AxisListType

IDX_CAP = 32768


# ----------------------------------------------------------------- host layout

def _host_layout(edge_index, n_nodes):
    src = np.asarray(edge_index[0]).astype(np.int64)
    dst = np.asarray(edge_index[1]).astype(np.int64)
    E = src.shape[0]
    deg = np.bincount(dst, minlength=n_nodes).astype(np.int64)
    order_e = np.argsort(dst, kind="stable")
    cum = np.cumsum(deg)
    estart = cum - deg

    bounds = [0]
    for c in range(1, NCORES):
        bounds.append(int(np.searchsorted(cum, E * c // NCORES)))
    bounds.append(n_nodes)
    n_loc = [bounds[c + 1] - bounds[c] for c in range(NCORES)]
    n_pad = max(((n + P - 1) // P) * P for n in n_loc)
    W = n_pad // P

    # ---- window chunks (for chunked AllGather); table is chunk-major:
    # row(c, w in chunk k, j) = 1 + 8*P*cuts[k] + c*P*(cuts[k+1]-cuts[k])
    #                             + (w-cuts[k])*P + j
    # uneven chunks: small final chunk so the last AllGather's exposed tail
    # (nothing left to overlap with) is short
    fr = [0.0, 0.25, 0.5, 0.75, 0.94, 1.0]
    cuts = sorted(set(int(round(W * f)) for f in fr))
    nch = len(cuts) - 1
    chunk_of_w = np.zeros(W, np.int64)
    for k in range(nch):
        chunk_of_w[cuts[k]:cuts[k + 1]] = k

    NG = NCORES * n_pad
    NTAB = NG + 2
    small = NTAB <= IDX_CAP
    base_b = 0 if small else NTAB - IDX_CAP

    # tabrow for each position (c*n_pad + w*P + j)
    pos_idx = np.arange(NCORES * n_pad)
    pc = pos_idx // n_pad
    pw = (pos_idx % n_pad) // P
    pj = pos_idx % P
    ck = chunk_of_w[pw]
    cuts_a = np.asarray(cuts)
    tabrow_of_pos = (1 + 8 * P * cuts_a[ck] +
                     pc * P * (cuts_a[ck + 1] - cuts_a[ck]) +
                     (pw - cuts_a[ck]) * P + pj)

    def _sort_cores(key2=None):
        nap = np.full(NCORES * n_pad, -1, np.int64)
        for c in range(NCORES):
            nodes = np.arange(bounds[c], bounds[c + 1])
            if key2 is None:
                o = np.argsort(-deg[nodes], kind="stable")
            else:
                o = np.lexsort((-key2[nodes], -(deg[nodes] // 4)))
            nap[c * n_pad: c * n_pad + len(nodes)] = nodes[o]
        return nap

    node_at_pos = _sort_cores()
    if not small:
        pos_of_node = np.full(n_nodes, -1, np.int64)
        v = node_at_pos >= 0
        pos_of_node[node_at_pos[v]] = np.nonzero(v)[0]
        tr_src = tabrow_of_pos[pos_of_node[src]]
        canA_e = tr_src <= (IDX_CAP - 1)
        canB_e = tr_src >= base_b
        nAo_n = np.bincount(dst[~canB_e], minlength=n_nodes)
        nBo_n = np.bincount(dst[~canA_e], minlength=n_nodes)
        node_at_pos = _sort_cores(key2=nAo_n - nBo_n)
    valid = node_at_pos >= 0
    pos_of_node = np.full(n_nodes, -1, np.int64)
    pos_of_node[node_at_pos[valid]] = np.nonzero(valid)[0]
    deg_at_pos = np.where(valid, deg[np.clip(node_at_pos, 0, None)], 0)
    tabrow_of_node = np.full(n_nodes, -1, np.int64)
    tabrow_of_node[node_at_pos[valid]] = tabrow_of_pos[valid]

    # ---- pass 1: per (c, w) sorted edge grids + eligibility counts
    grids = [[None] * W for _ in range(NCORES)]
    NAO = np.zeros((W, NCORES, P), np.int64)
    NBO = np.zeros((W, NCORES, P), np.int64)
    DD = np.zeros((W, NCORES, P), np.int64)
    for c in range(NCORES):
        for w in range(W):
            pos0 = c * n_pad + w * P
            nodes_w = node_at_pos[pos0: pos0 + P]
            deg_w = deg_at_pos[pos0: pos0 + P]
            DD[w, c] = deg_w
            d = int(deg_w.max()) if deg_w.size else 0
            if d == 0:
                grids[c][w] = (np.zeros((0, P), np.int64),
                               np.zeros((0, P), np.int64))
                continue
            est_w = np.where(nodes_w >= 0, estart[np.clip(nodes_w, 0, None)], 0)
            kk = np.arange(d)[:, None]
            spos = np.where(kk < deg_w[None, :], est_w[None, :] + kk, -1)
            eid = np.where(spos >= 0, order_e[np.clip(spos, 0, None)], -1)
            tr = np.where(eid >= 0,
                          tabrow_of_node[src[np.clip(eid, 0, None)]], -1)
            if small:
                key = np.where(eid >= 0, 0, 3)
            else:
                canA = (tr >= 0) & (tr <= (IDX_CAP - 1))
                canB = tr >= base_b
                key = np.where(eid < 0, 3,
                               np.where(~canB, 0, np.where(canA, 1, 2)))
            order = np.argsort(key, axis=0, kind="stable")
            eid_s = np.take_along_axis(eid, order, 0)
            tr_s = np.take_along_axis(tr, order, 0)
            grids[c][w] = (eid_s, tr_s)
            NAO[w, c] = (key == 0).sum(0)
            NBO[w, c] = (key == 2).sum(0)

    # ---- global schedule per window: (A_w, B_w)
    AB = []
    for w in range(W):
        dmax = int(DD[w].max())
        if small or dmax == 0:
            AB.append((dmax, 0))
            continue
        lowA = int(NAO[w].max())
        best = None
        for A_t in range(lowA, dmax + 1):
            B_t = int(np.maximum(NBO[w], DD[w] - A_t).max())
            cost = A_t + B_t
            if best is None or cost < best[0]:
                best = (cost, A_t, B_t)
        _, A_w, B_w = best
        AB.append((A_w, B_w))

    S = int(sum(a + b for a, b in AB))
    TOT_ROWS = P * S

    # per-window slab list: (is_b, slab_soff_in_window, slab_len)
    # gather groups subdivide slabs into chunks of <= G.
    win_slabs = []
    for (A_w, B_w) in AB:
        sl = []
        for is_b, T_k, off0 in ((0, A_w, 0), (1, B_w, A_w)):
            k0 = 0
            while k0 < T_k:
                s = min(SLAB, T_k - k0)
                sl.append((is_b, off0 + k0, s))
                k0 += s
        win_slabs.append(sl)

    # ---- pass 2: per-core data arrays (ea slab-major; gidx slot-major)
    ea_rows = np.empty((NCORES, TOT_ROWS), np.int64)
    gidx = np.empty((NCORES, P, 8 * S), np.int16)
    npad = np.empty((NCORES, P, W), np.float32)
    for c in range(NCORES):
        row_off = 0
        swin = 0
        for w in range(W):
            A_w, B_w = AB[w]
            T = A_w + B_w
            d_j = DD[w, c]
            npad[c, :, w] = (T - d_j).astype(np.float32)
            if T == 0:
                continue
            eid_s, tr_s = grids[c][w]
            d = eid_s.shape[0]
            a_j = np.minimum(d_j - NBO[w, c], A_w)
            rr = np.arange(d)[:, None]
            rows = np.where(rr < a_j[None, :], rr, A_w + rr - a_j[None, :])
            grid_eid = np.full((T, P), -1, np.int64)
            grid_tr = np.full((T, P), -1, np.int64)
            m = eid_s >= 0
            cols = np.broadcast_to(np.arange(P)[None, :], (d, P))
            grid_eid[rows[m], cols[m]] = eid_s[m]
            grid_tr[rows[m], cols[m]] = tr_s[m]
            loc = np.empty((T, P), np.int64)
            if A_w > 0:
                loc[:A_w] = np.where(grid_tr[:A_w] >= 0, grid_tr[:A_w], 0)
            if B_w > 0:
                loc[A_w:] = np.where(grid_tr[A_w:] >= 0,
                                     grid_tr[A_w:] - base_b, IDX_CAP - 1)
            assert loc.min() >= 0 and loc.max() < IDX_CAP
            blocks = loc.astype(np.int16).reshape(T, 8, 16).transpose(0, 2, 1)
            wrap = blocks.transpose(1, 0, 2).reshape(16, T * 8)
            gidx[c, 0:16, swin * 8: (swin + T) * 8] = wrap
            eT = grid_eid.T  # [P, T]
            for (_, soff, slen) in win_slabs[w]:
                ea_rows[c, row_off: row_off + P * slen] = eT[
                    :, soff: soff + slen].reshape(-1)
                row_off += P * slen
            swin += T
        assert row_off == TOT_ROWS and swin == S
    gidx[:, 16:, :] = np.tile(gidx[:, 0:16, :], (1, 7, 1))

    return dict(
        n_pad=n_pad, W=W, S=S, AB=AB, win_slabs=win_slabs, TOT_ROWS=TOT_ROWS,
        NTAB=NTAB, base_b=base_b, small=small, cuts=cuts, nch=nch,
        node_at_pos=node_at_pos, valid=valid, tabrow_of_pos=tabrow_of_pos,
        ea_rows=ea_rows, gidx=gidx, npad=npad, n_nodes=n_nodes,
    )


# ------------------------------------------------------------- device program

def _build_program(lay, betas, bout, ln_affine):
    n_pad, W, S = lay["n_pad"], lay["W"], lay["S"]
    AB, win_slabs = lay["AB"], lay["win_slabs"]
    TOT_ROWS = lay["TOT_ROWS"]
    NG = NCORES * n_pad
    NTAB = lay["NTAB"]
    base_b = lay["base_b"]
    cuts, nch = lay["cuts"], lay["nch"]

    nc = bacc.Bacc(None, target_bir_lowering=False, debug=False,
                   num_devices=NCORES,
                   dynamic_dma_scratch_size=SCRATCH,
                   num_swdge_queues=4)

    xtab = nc.dram_tensor("xtab", [NTAB, D], BF, kind="ExternalInput")
    ea_d = nc.dram_tensor("ea", [TOT_ROWS, D], BF, kind="ExternalInput")
    gidx_d = nc.dram_tensor("gidx", [P, 8 * S], I16, kind="ExternalInput")
    corr_d = nc.dram_tensor("corr", [P, L * W], FP, kind="ExternalInput")
    hin0_d = nc.dram_tensor("hin0", [P, W * D], FP, kind="ExternalInput")
    w1_d = nc.dram_tensor("W1", [L, D, 2 * D], BF, kind="ExternalInput")
    w2_d = nc.dram_tensor("W2", [L, 2 * D, D], BF, kind="ExternalInput")
    b1_d = nc.dram_tensor("b1", [L, 2 * D, 1], FP, kind="ExternalInput")
    b2_d = nc.dram_tensor("b2", [L, D, 1], FP, kind="ExternalInput")
    lnS_d = nc.dram_tensor("lnS", [P, L * D], FP, kind="ExternalInput")
    lnB_d = nc.dram_tensor("lnB", [P, L * D], FP, kind="ExternalInput")
    woutT_d = nc.dram_tensor("woutT", [P, D], FP, kind="ExternalInput")
    ebias_d = nc.dram_tensor("ebias", [P, L], FP, kind="ExternalInput")
    y_d = nc.dram_tensor("y", [n_pad, 1], FP, kind="ExternalOutput")

    hnloc = [nc.dram_tensor(f"hnloc{i}", [n_pad, D], BF) for i in range(L - 1)]
    tabAG = [
        nc.dram_tensor(f"tab{i}", [NTAB, D], BF, addr_space="Shared")
        for i in range(L - 1)
    ]

    dma_sems = [nc.alloc_semaphore(f"gsem{q}") for q in range(4)]

    with tile.TileContext(nc) as tc:
        with (
            tc.tile_pool(name="const", bufs=1) as constp,
            tc.tile_pool(name="persist", bufs=1) as persist,
            tc.tile_pool(name="slab", bufs=3) as slabp,
            tc.tile_pool(name="hsp", bufs=6 if G <= 8 else 4) as hsp,
            tc.tile_pool(name="edge", bufs=3 if G <= 8 else 2) as edgep,
            tc.tile_pool(name="node", bufs=2) as nodep,
            tc.tile_pool(name="ps_edge", bufs=2, space="PSUM") as ps_edge,
            tc.tile_pool(name="ps_node", bufs=2, space="PSUM") as ps_node,
        ):
            identf = constp.tile([P, P], FP, tag="identf")
            make_identity(nc, identf[:])
            ident = constp.tile([P, P], BF, tag="ident")
            nc.vector.tensor_copy(ident[:], identf[:])

            zr = constp.tile([1, D], BF, tag="zr")
            nc.vector.memset(zr[:], 0.0)
            for i in range(L - 1):
                nc.sync.dma_start(out=tabAG[i][0:1, :], in_=zr[:])
                nc.sync.dma_start(out=tabAG[i][NTAB - 1: NTAB, :], in_=zr[:])

            gidx_sb = persist.tile([P, 8 * S], I16, tag="gidx")
            nc.sync.dma_start(out=gidx_sb[:], in_=gidx_d[:, :])
            corr_sb = persist.tile([P, L * W], FP, tag="corr")
            nc.sync.dma_start(out=corr_sb[:], in_=corr_d[:, :])
            hin = persist.tile([P, W * D], FP, tag="hin")
            nc.sync.dma_start(out=hin[:], in_=hin0_d[:, :])
            hinb = persist.tile([P, W * D], BF, tag="hinb")
            h_sb = persist.tile([P, W * D], FP, tag="h")
            y_sb = persist.tile([P, W], FP, tag="ysb")
            su_all = persist.tile([P, W], FP, tag="su")
            mu_all = persist.tile([P, W], FP, tag="mu")
            ss_all = persist.tile([P, W], FP, tag="ss")
            inv_all = persist.tile([P, W], FP, tag="inv")

            lnS_sb = constp.tile([P, L * D], FP, tag="lnS")
            lnB_sb = constp.tile([P, L * D], FP, tag="lnB")
            nc.sync.dma_start(out=lnS_sb[:], in_=lnS_d[:, :])
            nc.sync.dma_start(out=lnB_sb[:], in_=lnB_d[:, :])
            woutT_sb = constp.tile([P, D], FP, tag="wout")
            nc.sync.dma_start(out=woutT_sb[:], in_=woutT_d[:, :])
            ebias_sb = constp.tile([P, L], FP, tag="ebias")
            nc.sync.dma_start(out=ebias_sb[:], in_=ebias_d[:, :])

            tabs = [xtab] + tabAG
            qrot = 0

            for l in range(L):
                table = tabs[l]
                w1a = constp.tile([P, P], BF, tag="w1a")
                w1b = constp.tile([P, P], BF, tag="w1b")
                w2a = constp.tile([P, P], BF, tag="w2a")
                w2b = constp.tile([P, P], BF, tag="w2b")
                nc.sync.dma_start(out=w1a[:], in_=w1_d[l, :, 0:P])
                nc.sync.dma_start(out=w1b[:], in_=w1_d[l, :, P: 2 * P])
                nc.sync.dma_start(out=w2a[:], in_=w2_d[l, 0:P, :])
                nc.sync.dma_start(out=w2b[:], in_=w2_d[l, P: 2 * P, :])
                b1a = constp.tile([P, 1], FP, tag="b1a")
                b1b = constp.tile([P, 1], FP, tag="b1b")
                b2c = constp.tile([P, 1], FP, tag="b2c")
                nc.sync.dma_start(out=b1a[:], in_=b1_d[l, 0:P, :])
                nc.sync.dma_start(out=b1b[:], in_=b1_d[l, P: 2 * P, :])
                nc.sync.dma_start(out=b2c[:], in_=b2_d[l, :, :])

                srcA = table[0: min(IDX_CAP, NTAB), :]
                srcB = table[base_b:NTAB, :]
                swin = 0
                row_off = 0
                last = l == L - 1
                for w in range(W):
                    A_w, B_w = AB[w]
                    T = A_w + B_w
                    wsl = slice(w * D, (w + 1) * D)
                    if T > 0:
                        acc_ps = ps_edge.tile([P, 2 * D], FP, tag="acc")
                        tdone = 0
                        for (is_b, soff, slen) in win_slabs[w]:
                            # ea slab: [P, slen*D] bf16, contiguous per lane
                            easl = slabp.tile([P, SLAB * D], BF, tag="easl")
                            nc.sync.dma_start(
                                out=easl[:, 0: slen * D],
                                in_=ea_d[row_off: row_off + P * slen, :]
                                .rearrange("(p q) d -> p (q d)", p=P),
                            )
                            row_off += P * slen
                            k0 = 0
                            while k0 < slen:
                                g = min(G, slen - k0)
                                sg = swin + soff + k0
                                hs = hsp.tile([P, G * D], BF, tag="hs")
                                q = qrot % 4
                                if GPREP:
                                    nc.gpsimd.dma_gather(
                                        hs[:, 0: g * D].rearrange(
                                            "p (q d) -> p q d", d=D),
                                        srcB if is_b else srcA,
                                        gidx_sb[:, sg * 8: (sg + g) * 8],
                                        g * P,
                                        g * P,
                                        D,
                                        queue_num=q,
                                        prepare_only=True,
                                        sem=dma_sems[q],
                                    )
                                    nc.gpsimd.trigger_dma(
                                        count=None, queue_num=q)
                                else:
                                    nc.gpsimd.dma_gather(
                                        hs[:, 0: g * D].rearrange(
                                            "p (q d) -> p q d", d=D),
                                        srcB if is_b else srcA,
                                        gidx_sb[:, sg * 8: (sg + g) * 8],
                                        g * P,
                                        g * P,
                                        D,
                                        queue_num=q,
                                    )
                                qrot += 1
                                tb = edgep.tile([P, G * D], BF, tag="tb")
                                msg = edgep.tile([P, G * D], BF, tag="msg")
                                ppm = edgep.tile([P, G * 2 * D], BF, tag="ppm")
                                # t = ea + h_src (DVE); msg = relu(t) (ACT)
                                nc.vector.tensor_tensor(
                                    out=tb[:, 0: g * D],
                                    in0=easl[:, k0 * D: (k0 + g) * D],
                                    in1=hs[:, 0: g * D], op=ALU.add)
                                nc.scalar.activation(
                                    msg[:, 0: g * D], tb[:, 0: g * D],
                                    AF.Relu)
                                pv = ppm[:].rearrange(
                                    "p (q dd) -> p q dd", dd=2 * D)
                                mv = msg[:].rearrange(
                                    "p (q d) -> p q d", d=D)
                                # e = exp(beta*msg + beta*eps)   (ACT)
                                nc.scalar.activation(
                                    pv[:, 0:g, 0:D], mv[:, 0:g, :],
                                    AF.Exp, scale=float(betas[l]),
                                    bias=ebias_sb[:, l: l + 1],
                                )
                                # p = e * msg   (DVE)
                                nc.vector.tensor_tensor(
                                    out=pv[:, 0:g, D: 2 * D],
                                    in0=pv[:, 0:g, 0:D],
                                    in1=mv[:, 0:g, :], op=ALU.mult)
                                for gi in range(g):
                                    nc.tensor.matmul(
                                        acc_ps[:],
                                        lhsT=ident[:],
                                        rhs=ppm[:, gi * 2 * D: (gi + 1) * 2 * D],
                                        start=(tdone + gi == 0),
                                        stop=(tdone + gi == T - 1),
                                    )
                                tdone += g
                                k0 += g

                    # ---------------- node phase A for window w
                    z = nodep.tile([P, D], FP, tag="z")
                    if T > 0:
                        denc = nodep.tile([P, D], FP, tag="denc")
                        nc.vector.tensor_scalar(
                            out=denc[:], in0=acc_ps[:, 0:D],
                            scalar1=corr_sb[:, l * W + w: l * W + w + 1],
                            scalar2=1e-6, op0=ALU.subtract, op1=ALU.max)
                        rec = nodep.tile([P, D], FP, tag="rec")
                        nc.vector.reciprocal(rec[:], denc[:])
                        nc.vector.tensor_tensor(
                            out=z[:], in0=acc_ps[:, D: 2 * D], in1=rec[:],
                            op=ALU.mult)
                        nc.vector.tensor_tensor(
                            out=z[:], in0=z[:], in1=hin[:, wsl], op=ALU.add)
                    else:
                        nc.vector.tensor_copy(z[:], hin[:, wsl])

                    zb = nodep.tile([P, D], BF, tag="zb")
                    nc.scalar.activation(zb[:], z[:], AF.Identity)
                    zT_ps = ps_node.tile([P, D], BF, tag="tp")
                    nc.tensor.transpose(zT_ps[:], zb[:], ident[:])
                    zT = nodep.tile([P, D], BF, tag="zT")
                    nc.scalar.activation(zT[:], zT_ps[:], AF.Identity)
                    y1_ps = ps_node.tile([P, 2 * D], FP, tag="y1")
                    nc.tensor.matmul(y1_ps[:, 0:D], lhsT=w1a[:], rhs=zT[:],
                                     start=True, stop=True)
                    nc.tensor.matmul(y1_ps[:, D: 2 * D], lhsT=w1b[:], rhs=zT[:],
                                     start=True, stop=True)
                    r1 = nodep.tile([P, 2 * D], BF, tag="r1")
                    nc.scalar.activation(r1[:, 0:D], y1_ps[:, 0:D],
                                         AF.Relu, bias=b1a[:, 0:1])
                    nc.scalar.activation(r1[:, D: 2 * D], y1_ps[:, D: 2 * D],
                                         AF.Relu, bias=b1b[:, 0:1])
                    y2_ps = ps_node.tile([P, D], FP, tag="y2")
                    nc.tensor.matmul(y2_ps[:], lhsT=w2a[:], rhs=r1[:, 0:D],
                                     start=True, stop=False)
                    nc.tensor.matmul(y2_ps[:], lhsT=w2b[:], rhs=r1[:, D: 2 * D],
                                     start=False, stop=True)
                    y2b = nodep.tile([P, D], BF, tag="y2b")
                    nc.scalar.activation(y2b[:], y2_ps[:], AF.Identity,
                                         bias=b2c[:, 0:1])
                    hn_ps = ps_node.tile([P, D], BF, tag="tp")
                    nc.tensor.transpose(hn_ps[:], y2b[:], ident[:])
                    if l == 0:
                        nc.vector.tensor_copy(h_sb[:, wsl], hn_ps[:])
                    else:
                        hn32 = nodep.tile([P, D], FP, tag="hn32")
                        nc.scalar.activation(hn32[:], hn_ps[:], AF.Identity)
                        nc.vector.tensor_tensor(
                            out=h_sb[:, wsl], in0=h_sb[:, wsl], in1=hn32[:],
                            op=ALU.add)

                    # LN stats: mean and sum of squares
                    hw = h_sb[:, wsl]
                    nc.vector.reduce_sum(
                        out=su_all[:, w: w + 1], in_=hw, axis=AX.X)
                    nc.vector.tensor_scalar(
                        out=mu_all[:, w: w + 1], in0=su_all[:, w: w + 1],
                        scalar1=1.0 / D, scalar2=None, op0=ALU.mult)
                    sq = nodep.tile([P, D], FP, tag="sq")
                    nc.vector.tensor_tensor(out=sq[:], in0=hw, in1=hw,
                                            op=ALU.mult)
                    nc.vector.reduce_sum(
                        out=ss_all[:, w: w + 1], in_=sq[:], axis=AX.X)
                    swin += T

                    # ---------------- chunk boundary: LN finish (+ AllGather)
                    kk = next((k for k in range(nch) if cuts[k + 1] == w + 1),
                              None)
                    if kk is None:
                        continue
                    w0, w1 = cuts[kk], cuts[kk + 1]
                    nw = w1 - w0
                    csl = slice(w0, w1)
                    # var = ss/D - mu^2 ;  inv = rsqrt(var + LN_EPS)
                    m2 = nodep.tile([P, W], FP, tag="m2")
                    rsin = nodep.tile([P, W], FP, tag="rsin")
                    nc.vector.tensor_tensor(
                        out=m2[:, csl], in0=mu_all[:, csl],
                        in1=mu_all[:, csl], op=ALU.mult)
                    nc.vector.tensor_scalar(
                        out=rsin[:, csl], in0=ss_all[:, csl],
                        scalar1=1.0 / D, scalar2=None, op0=ALU.mult)
                    nc.vector.tensor_tensor(
                        out=rsin[:, csl], in0=rsin[:, csl],
                        in1=m2[:, csl], op=ALU.subtract)
                    nc.vector.tensor_scalar(
                        out=rsin[:, csl], in0=rsin[:, csl],
                        scalar1=LN_EPS, scalar2=None, op0=ALU.add)
                    nc.scalar.activation(rsin[:, csl], rsin[:, csl], AF.Sqrt)
                    nc.vector.reciprocal(inv_all[:, csl], rsin[:, csl])
                    for wv in range(w0, w1):
                        wsl2 = slice(wv * D, (wv + 1) * D)
                        hnorm = nodep.tile([P, D], FP, tag="hnorm")
                        nc.vector.tensor_scalar(
                            out=hnorm[:], in0=h_sb[:, wsl2],
                            scalar1=mu_all[:, wv: wv + 1],
                            scalar2=inv_all[:, wv: wv + 1],
                            op0=ALU.subtract, op1=ALU.mult)
                        if ln_affine[l]:
                            nc.vector.tensor_tensor(
                                out=hnorm[:], in0=hnorm[:],
                                in1=lnS_sb[:, l * D: (l + 1) * D], op=ALU.mult)
                            nc.vector.tensor_tensor(
                                out=hnorm[:], in0=hnorm[:],
                                in1=lnB_sb[:, l * D: (l + 1) * D], op=ALU.add)
                        if not last:
                            nc.vector.tensor_scalar(
                                out=hin[:, wsl2], in0=hnorm[:],
                                scalar1=0.0, scalar2=None, op0=ALU.max)
                            nc.vector.tensor_copy(hinb[:, wsl2], hin[:, wsl2])
                        else:
                            hnf = nodep.tile([P, D], FP, tag="hnf")
                            nc.vector.tensor_scalar(
                                out=hnf[:], in0=hnorm[:],
                                scalar1=0.0, scalar2=None, op0=ALU.max)
                            yw = nodep.tile([P, D], FP, tag="yw")
                            nc.vector.tensor_tensor(
                                out=yw[:], in0=hnf[:], in1=woutT_sb[:, :],
                                op=ALU.mult)
                            nc.vector.reduce_sum(
                                out=y_sb[:, wv: wv + 1], in_=yw[:], axis=AX.X)
                    if not last:
                        nc.sync.dma_start(
                            out=hnloc[l][P * w0: P * w1, :].rearrange(
                                "(w p) d -> p w d", p=P),
                            in_=hinb[:, w0 * D: w1 * D].rearrange(
                                "p (w d) -> p w d", d=D),
                        )
                        r0 = 1 + 8 * P * w0
                        rk = P * nw
                        nc.gpsimd.collective_compute(
                            "AllGather",
                            ALU.bypass,
                            replica_groups=[list(range(NCORES))],
                            ins=[hnloc[l][P * w0: P * w1, :]],
                            outs=[tabAG[l][r0: r0 + 8 * rk, :]],
                        )

            nc.vector.tensor_scalar(
                out=y_sb[:], in0=y_sb[:], scalar1=float(bout), scalar2=None,
                op0=ALU.add)
            nc.sync.dma_start(
                out=y_d[:, :].rearrange("(w p) o -> p w o", p=P),
                in_=y_sb[:].rearrange("p (w o) -> p w o", o=1),
            )

    nc.compile()
    return nc


# ------------------------------------------------------------------- inputs

def _build_in_maps(inputs, lay):
    x = np.ascontiguousarray(np.asarray(inputs["x"], np.float32))
    ea = np.ascontiguousarray(np.asarray(inputs["edge_attr"], np.float32))
    W1 = np.ascontiguousarray(np.asarray(inputs["W1"], np.float32))
    b1 = np.asarray(inputs["b1"], np.float32).reshape(L, 2 * D)
    W2 = np.ascontiguousarray(np.asarray(inputs["W2"], np.float32))
    b2 = np.asarray(inputs["b2"], np.float32).reshape(L, D, 1)
    beta = np.asarray(inputs["beta"], np.float32)
    ln_scale = np.asarray(inputs["ln_scale"], np.float32)
    ln_bias = np.asarray(inputs["ln_bias"], np.float32)
    lnf_scale = np.asarray(inputs["lnf_scale"], np.float32)
    lnf_bias = np.asarray(inputs["lnf_bias"], np.float32)
    Wout = np.asarray(inputs["Wout"], np.float32)

    n_pad, W, S = lay["n_pad"], lay["W"], lay["S"]
    NTAB = lay["NTAB"]
    node_at_pos, valid = lay["node_at_pos"], lay["valid"]
    tabrow_of_pos = lay["tabrow_of_pos"]

    xtab = np.zeros((NTAB, D), ml_dtypes.bfloat16)
    xtab[tabrow_of_pos[valid]] = x[node_at_pos[valid]].astype(
        ml_dtypes.bfloat16)

    # eps folded into b1: b1' = b1 + EPS * colsum(W1)
    b1eps = (b1 + np.float32(EPS) * W1.sum(axis=1)).reshape(L, 2 * D, 1)

    lnS = np.zeros((L, D), np.float32)
    lnB = np.zeros((L, D), np.float32)
    for l in range(L - 1):
        lnS[l] = ln_scale[l + 1]
        lnB[l] = ln_bias[l + 1]
    lnS[L - 1] = lnf_scale
    lnB[L - 1] = lnf_bias
    ln_affine = [
        not (np.all(lnS[l] == 1.0) and np.all(lnB[l] == 0.0)) for l in range(L)
    ]
    lnS_rep = np.ascontiguousarray(np.tile(lnS.reshape(1, L * D), (P, 1)))
    lnB_rep = np.ascontiguousarray(np.tile(lnB.reshape(1, L * D), (P, 1)))
    wout_rep = np.ascontiguousarray(np.tile(Wout.reshape(1, D), (P, 1)))

    # pad-lane exp value, with device bf16 rounding: e_pad = bf16(exp(b*eps))
    e_pad = np.float32(
        np.asarray(np.exp(beta * np.float32(EPS)), ml_dtypes.bfloat16))
    ebias = np.tile((beta * np.float32(EPS)).reshape(1, L), (P, 1)).astype(
        np.float32)

    in_maps = []
    for c in range(NCORES):
        rows = lay["ea_rows"][c]
        ea_c = ea[np.clip(rows, 0, None)].astype(ml_dtypes.bfloat16)
        ea_c[rows < 0] = 0.0
        corr = np.zeros((P, L * W), np.float32)
        for l in range(L):
            corr[:, l * W: (l + 1) * W] = lay["npad"][c] * e_pad[l]
        hin0 = (
            x[np.clip(node_at_pos[c * n_pad: (c + 1) * n_pad], 0, None)]
            * valid[c * n_pad: (c + 1) * n_pad][:, None]
        ).reshape(W, P, D).transpose(1, 0, 2).reshape(P, W * D)
        in_maps.append(
            {
                "xtab": xtab,
                "ea": np.ascontiguousarray(ea_c),
                "gidx": np.ascontiguousarray(lay["gidx"][c]),
                "corr": corr,
                "hin0": np.ascontiguousarray(hin0.astype(np.float32)),
                "W1": W1.astype(ml_dtypes.bfloat16),
                "W2": W2.astype(ml_dtypes.bfloat16),
                "b1": np.ascontiguousarray(b1eps),
                "b2": np.ascontiguousarray(b2),
                "lnS": lnS_rep,
                "lnB": lnB_rep,
                "woutT": wout_rep,
                "ebias": ebias,
            }
        )
    meta = dict(
        betas=[float(b) for b in beta],
        bout=float(np.asarray(inputs["bout"]).reshape(-1)[0]),
        ln_affine=ln_affine,
    )
    return in_maps, meta


_CACHE = {}


def _get_program(inputs):
    edge_index = np.asarray(inputs["edge_index"])
    key = hash(
        (
            edge_index.tobytes(),
            np.asarray(inputs["beta"], np.float32).tobytes(),
            np.asarray(inputs["bout"], np.float32).tobytes(),
            np.asarray(inputs["ln_scale"], np.float32).tobytes(),
            np.asarray(inputs["ln_bias"], np.float32).tobytes(),
            np.asarray(inputs["lnf_scale"], np.float32).tobytes(),
            np.asarray(inputs["lnf_bias"], np.float32).tobytes(),
        )
    )
    if key not in _CACHE:
        n_nodes = np.asarray(inputs["x"]).shape[0]
        lay = _host_layout(edge_index, n_nodes)
        in_maps, meta = _build_in_maps(inputs, lay)
        nc = _build_program(lay, meta["betas"], meta["bout"], meta["ln_affine"])
        _CACHE[key] = (nc, lay)
        return nc, lay, in_maps
    nc, lay = _CACHE[key]
    in_maps, _ = _build_in_maps(inputs, lay)
    return nc, lay, in_maps


def kernel(**inputs) -> np.ndarray:
    nc, lay, in_maps = _get_program(inputs)
    res = run_bass_kernel_spmd(nc, in_maps, list(range(NCORES)))
    results = res.results
    ys = np.concatenate([results[c]["y"] for c in range(NCORES)], axis=0)
    out = np.zeros((lay["n_nodes"], 1), np.float32)
    valid = lay["valid"]
    out[lay["node_at_pos"][valid]] = ys[valid]
    return out


NCORES_EXPORT = NCORES
